# revision 22
# baseline (speedup 1.0000x reference)
"""CSPAttention Trainium2 kernel: 8-way SPMD (batch x seq-half), no collectives.

Sharding: core = b*2 + half; each core computes 1024 query rows of batch b
against the full 2048-token K/V of that batch.  Host side does layout
marshalling only (transposes + fp32->bf16 casts + tiny bias matvec); all
large FLOPs run on device.

Device plan (per core, bf16 matmuls, fp32 PSUM accumulate):
  phase1: Q/K projections dc-outer over per-dc-sliced input DMAs (PE starts
          ~2us in); depthwise conv runs on DVE (3 shifted mul-adds with
          per-partition taps) in parallel.
  attn:   per head, per kv-chunk kc: S.T = K_chunk.T @ Q (2 matmuls into one
          2-bank PSUM tile), one Exp ACTIVATE over [128,1024] (scale=1/8
          folded) -> bf16 SBUF, PV accumulates [denom|O] over kc.  ACT does
          ONLY exp in this phase (its floor, ~1.33us/chunk); the PE slack is
          filled by a drip stream emitted right after each exp: V projection,
          then Wo_conv, then the conv-half of Wf (accumulated into the
          residual via DVE).  Drips lead their consumers so the in-order PE
          queue never inverts a dependency.
  norm:   reciprocal_approx_fast on the denominator rows + DVE multiply.
  tail:   Wo_attn (ldweights reused across both token halves), then Wf over
          the attn features only; the residual (+bf+Wf_conv partial) enters
          the same PSUM via an identity matmul so LayerNorm stats/apply read
          PSUM directly.  LN spread over DVE (stats) / ACT (apply) /
          GpSimd (beta add + out DMA with bf16->f32 cast).
"""

import sys

sys.path.insert(0, '/opt/trn_rl_repo')

import numpy as np
import ml_dtypes

import concourse.bass as bass
import concourse.tile as tile
from concourse import bacc, mybir

F32 = mybir.dt.float32
BF16 = mybir.dt.bfloat16
BF = ml_dtypes.bfloat16

B, L, D = 4, 2048, 1024
DA = 512          # attention channels
DC = 512          # conv channels
H = 8             # heads
HD = 64           # head dim
N_CORES = 8
TQ = 1024         # query rows per core
TK = 2048         # kv rows per core
NTQ = TQ // 512   # 512-token tiles of queries
NTK = TK // 512
NQC = TQ // 128   # 128-token query chunks
NKC = TK // 128   # 128-token kv chunks
NDA = DA // 128
ND = D // 128
LN_EPS = 1e-5

Identity = mybir.ActivationFunctionType.Identity
Exp = mybir.ActivationFunctionType.Exp
Sqrt = mybir.ActivationFunctionType.Sqrt
AOp = mybir.AluOpType


def _chunked(t, nch, w, col0=0, ncol=None):
    """DRAM [nch*128, w] -> SBUF-layout AP [128, nch, ncol] starting at col0."""
    if ncol is None:
        ncol = w
    return bass.AP(tensor=t, offset=col0,
                   ap=[[w, 128], [128 * w, nch], [1, ncol]])


def _feat_bias(t, nch):
    """DRAM [nch*128] -> SBUF [128, nch] feature-major bias."""
    return bass.AP(tensor=t, offset=0, ap=[[1, 128], [128, nch]])


def _bcast(t, n):
    """DRAM [n] -> [128, n] partition broadcast."""
    return bass.AP(tensor=t, offset=0, ap=[[0, 128], [1, n]])


def _rows(t, w, r0, nr, c0=0, nc_=None):
    """DRAM [*, w] rows r0:r0+nr, cols c0:c0+nc_ -> SBUF [nr, nc_]."""
    if nc_ is None:
        nc_ = w
    return bass.AP(tensor=t, offset=r0 * w + c0, ap=[[w, nr], [1, nc_]])


def build_nc(reps: int = 1):
    nc = bacc.Bacc('TRN2', target_bir_lowering=False, debug=False,
                   num_devices=N_CORES)

    def din(name, shape, dt):
        return nc.dram_tensor(name, list(shape), dt, kind='ExternalInput')

    t = {n: din(n, s, dt) for n, s, dt in [
        ('qaT', [DA, TQ], BF16), ('qcT', [DC, TQ + 2], BF16),
        ('qres', [TQ, D], BF16),
        ('kT', [DA, TK], BF16), ('vT', [DA, TK], BF16),
        ('wqT', [DA, DA], BF16), ('wkT', [DA, DA], BF16),
        ('wvT', [DA, DA], BF16), ('woaT', [DA, DA], BF16),
        ('wocT', [DC, DC], BF16), ('wfT', [D, D], BF16),
        ('cw', [DC, 3], F32), ('bq', [DA], F32), ('bk', [DA], F32),
        ('bv', [DA], F32), ('boa', [DA], F32), ('cbe', [DC], F32),
        ('gamma', [D], F32),
        ('beta', [D], F32), ('ident', [128, 128], BF16)]}
    t['out'] = nc.dram_tensor('out', [TQ, D], F32, kind='ExternalOutput')

    with tile.TileContext(nc) as tc:
        for rep in range(reps):
            _build_rep(nc, tc, rep, t)
    nc.compile()
    return nc


def _build_rep(nc, tc, rep, t):
    R = f'r{rep}_'
    with tc.tile_pool(name=R + 'persist', bufs=1) as pp:
        fT_s = pp.tile([128, ND, TQ], BF16)       # concat features, fm
        q_s = pp.tile([128, NDA, TQ], BF16)
        k_s = pp.tile([128, NDA, TK], BF16)
        vaug = pp.tile([128, NKC, H, 128], BF16)  # [ones(64) | V(64)]
        o_s = pp.tile([128, NDA, TQ], BF16)       # normalized attn out, fm
        cdw_s = pp.tile([128, NDA, TQ], BF16)     # depthwise conv out, fm
        cw_s = pp.tile([128, NDA, 3], F32)        # conv taps, feature-major
        cbe_s = pp.tile([128, NDA], F32)          # conv effective bias
        resbf = pp.tile([128, NQC, D], BF16)      # res + bf (+Wf_conv), tm
        wf_s = pp.tile([128, ND, D], BF16)        # prefetched tail weights
        woa_s = pp.tile([128, NDA, DA], BF16)
        wocT_s = pp.tile([128, NDA, DC], BF16)
        wv_s = pp.tile([128, NDA, DA], BF16)
        v_in = pp.tile([128, NDA, TK], BF16)
        wk_s = pp.tile([128, NDA, DA], BF16)
        bk_s = pp.tile([128, NDA], F32)
        k_in = pp.tile([128, NDA, TK], BF16)
        qcT_s = pp.tile([128, NDA, TQ + 2], BF16)
        boa_s = pp.tile([128, NDA], F32)
        bv_fm = pp.tile([128, NDA], F32)
        ga_bc = pp.tile([128, D], BF16)
        be_bc = pp.tile([128, D], BF16)
        id_s = pp.tile([128, 128], BF16)
        eps_s = pp.tile([128, 1], F32)
        nc.vector.memset(eps_s, LN_EPS)

        # preload the exp table set while DMAs stream
        warm = pp.tile([128, 1], F32)
        nc.vector.memset(warm, 0.0)
        nc.scalar.activation(warm, warm, Exp)


        # ---- everything non-critical queues behind on gpsimd (FIFO ring
        # keeps it off the HBM pipe while Q/K/V stream) ----
        nc.gpsimd.dma_start(out=bv_fm, in_=_feat_bias(t['bv'], NDA))
        nc.gpsimd.dma_start(out=cw_s, in_=_chunked(t['cw'], NDA, 3))
        nc.gpsimd.dma_start(out=cbe_s, in_=_feat_bias(t['cbe'], NDA))
        nc.gpsimd.dma_start(out=boa_s, in_=_feat_bias(t['boa'], NDA))
        nc.gpsimd.dma_start(out=bk_s, in_=_feat_bias(t['bk'], NDA))
        nc.gpsimd.dma_start(out=wv_s, in_=_chunked(t['wvT'], NDA, DA))
        nc.gpsimd.dma_start(out=wv_s, in_=_chunked(t['wvT'], NDA, DA))
        # whole-tile contiguous memset (4x DVE mode); V copies overwrite
        # the V halves, the ones halves remain
        nc.vector.memset(vaug[:, :, :, :], 1.0)

        # ------------- Q/K projections (sync-queue DMAs, dc-sliced) --------
        with tc.tile_pool(name=R + 'projw', bufs=1) as wpj, \
             tc.tile_pool(name=R + 'qin', bufs=1) as qip:
            bq_s = wpj.tile([128, NDA], F32, tag='bq')
            nc.gpsimd.dma_start(out=bq_s, in_=_feat_bias(t['bq'], NDA))
            wq_s = wpj.tile([128, NDA, DA], BF16, tag='wq')
            nc.scalar.dma_start(out=wq_s, in_=_chunked(t['wqT'], NDA, DA))
            qa_in = qip.tile([128, NDA, TQ], BF16, tag='qa')
            nc.sync.dma_start(out=qa_in, in_=_chunked(t['qaT'], NDA, TQ))
            nc.sync.dma_start(out=wk_s, in_=_chunked(t['wkT'], NDA, DA))
            # wq + kh0 lead the scalar queue; kh1 + vT1 behind; vT0 on sync
            nc.scalar.dma_start(out=k_in[:, :, 0:1024],
                                in_=_chunked(t['kT'], NDA, TK, 0, 1024))
            for vq in range(2):
                nc.sync.dma_start(
                    out=v_in[:, :, vq * 512:(vq + 1) * 512],
                    in_=_chunked(t['vT'], NDA, TK, vq * 512, 512))
            nc.scalar.dma_start(out=k_in[:, :, 1024:2048],
                              in_=_chunked(t['kT'], NDA, TK, 1024, 1024))
            nc.scalar.dma_start(out=v_in[:, :, 1024:2048],
                                in_=_chunked(t['vT'], NDA, TK, 1024, 1024))
            # Bulk loads are gated behind the critical-path inputs (dummy
            # reads stall the gpsimd FIFO) so they don't steal HBM bandwidth
            # from qa/kT/vT during the first ~25us.
            gate = wpj.tile([128, 1], BF16, tag='gate')
            nc.gpsimd.tensor_copy(out=gate, in_=qa_in[:, NDA - 1, TQ - 1:TQ])
            nc.gpsimd.dma_start(out=qcT_s,
                                in_=_chunked(t['qcT'], NDA, TQ + 2))
            nc.gpsimd.tensor_copy(out=gate, in_=k_in[:, NDA - 1, 1023:1024])
            nc.gpsimd.dma_start(out=wocT_s, in_=_chunked(t['wocT'], NDA, DC))
            nc.gpsimd.tensor_copy(out=gate,
                                  in_=v_in[:, NDA - 1, 1023:1024])
            nc.gpsimd.dma_start(out=resbf, in_=_chunked(t['qres'], NQC, D))
            nc.gpsimd.dma_start(out=wf_s, in_=_chunked(t['wfT'], ND, D))
            nc.gpsimd.dma_start(out=woa_s, in_=_chunked(t['woaT'], NDA, DA))
            nc.gpsimd.dma_start(out=ga_bc, in_=_bcast(t['gamma'], D))
            nc.gpsimd.dma_start(out=be_bc, in_=_bcast(t['beta'], D))
            nc.gpsimd.dma_start(out=id_s, in_=_rows(t['ident'], 128, 0, 128))

            with tc.tile_pool(name=R + 'ps_proj', bufs=1,
                              space='PSUM') as ps_proj:
                # Q: dc-outer accumulation so matmuls start on the 1st slice
                pq = [ps_proj.tile([128, 512], F32, name=f'pq{j}',
                                   tag=f'projp{j}') for j in range(8)]
                for dc in range(NDA):
                    for oc in range(NDA):
                        for tt in range(NTQ):
                            nc.tensor.matmul(
                                pq[oc * 2 + tt][:, :],
                                wq_s[:, dc, oc * 128:(oc + 1) * 128],
                                qa_in[:, dc, tt * 512:(tt + 1) * 512],
                                start=(dc == 0), stop=(dc == NDA - 1))
                for oc in range(NDA):
                    for tt in range(NTQ):
                        if oc < 2:
                            nc.scalar.activation(
                                q_s[:, oc, tt * 512:(tt + 1) * 512],
                                pq[oc * 2 + tt][:, :], Identity,
                                bias=bq_s[:, oc:oc + 1])
                        else:
                            nc.vector.tensor_scalar_add(
                                q_s[:, oc, tt * 512:(tt + 1) * 512],
                                pq[oc * 2 + tt][:, :], bq_s[:, oc:oc + 1])

                # K: dc-outer, tg0 only (tg1 is emitted inside head 0
                # once its kT half has streamed in); copies on DVE except
                # oc2 (heads 4-5) which rides gpsimd
                for tg in range(1):
                    pk = [ps_proj.tile([128, 512], F32, name=f'pk{tg}{j}',
                                       tag=f'projp{j}') for j in range(8)]
                    for dc in range(NDA):
                        for oc in range(NDA):
                            for tt in range(2):
                                nc.tensor.matmul(
                                    pk[oc * 2 + tt][:, :],
                                    wk_s[:, dc, oc * 128:(oc + 1) * 128],
                                    k_in[:, dc, (tg * 2 + tt) * 512:
                                         (tg * 2 + tt + 1) * 512],
                                    start=(dc == 0), stop=(dc == NDA - 1))
                    for oc in range(NDA):
                        for tt in range(2):
                            dst = k_s[:, oc, (tg * 2 + tt) * 512:
                                      (tg * 2 + tt + 1) * 512]
                            nc.vector.tensor_scalar_add(
                                dst, pk[oc * 2 + tt][:, :],
                                bk_s[:, oc:oc + 1])

                # V projection chunks 0..7 (vT cols 0:1024), static
                for kc in range(8):
                    pv = ps_proj.tile([128, 512], F32, name=f'pvs{kc}',
                                      tag=f'projp{kc % 2}')
                    for dc in range(NDA):
                        nc.tensor.matmul(
                            pv[:, :],
                            v_in[:, dc, kc * 128:(kc + 1) * 128],
                            wv_s[:, dc, :],
                            start=(dc == 0), stop=(dc == NDA - 1))
                    nc.vector.tensor_copy(
                        out=vaug[:, kc, :, 64:128],
                        in_=pv[:].rearrange('p (h x) -> p h x', h=H))


        # ---------------- attention (+ V/Wo_conv/Wf_conv drip) --------------
        with tc.tile_pool(name=R + 'pst', bufs=2, space='PSUM') as ps_s, \
             tc.tile_pool(name=R + 'pso', bufs=1, space='PSUM') as ps_o, \
             tc.tile_pool(name=R + 'psd', bufs=2, space='PSUM') as ps_d, \
             tc.tile_pool(name=R + 'pwork', bufs=5) as wp, \
             tc.tile_pool(name=R + 'rnorm', bufs=1) as rp:

            # depthwise conv ops (DVE), interleaved into the drip stream so
            # they sit behind the V copies in the DVE FIFO, not ahead
            def dw_ops():
                for c in range(NDA):
                    nc.vector.tensor_scalar_mul(
                        cdw_s[:, c, :], qcT_s[:, c, 0:TQ], cw_s[:, c, 0:1])
                    yield
                for k in (1, 2):
                    for c in range(NDA):
                        nc.vector.scalar_tensor_tensor(
                            cdw_s[:, c, :], qcT_s[:, c, k:k + TQ],
                            cw_s[:, c, k:k + 1],
                            cdw_s[:, c, :], AOp.mult, AOp.add)
                        yield

            dw_gen = dw_ops()

            # drip generator: yields after each emitted matmul.  Emitted
            # right after each exp so drips always precede their consumers
            # in the in-order PE stream.
            def drip_steps():
                # V projection: token-major into vaug[kv, kc, h, 64:128].
                # bv is NOT added here: sum_kv p*(V+bv) = PV + denom*bv, so
                # the bias is applied after normalization instead.
                for kc in range(8, NKC):
                    pv = ps_d.tile([128, 512], F32, tag='vd')
                    for dc in range(NDA):
                        nc.tensor.matmul(
                            pv[:, :],
                            v_in[:, dc, kc * 128:(kc + 1) * 128],
                            wv_s[:, dc, :],
                            start=(dc == 0), stop=(dc == NDA - 1),
                            skip_group_check=True)
                        yield
                    nc.vector.tensor_copy(
                        out=vaug[:, kc, :, 64:128],
                        in_=pv[:].rearrange('p (h x) -> p h x', h=H))
                    for _ in range(2 if kc >= 12 else 1):
                        try:
                            next(dw_gen)
                        except StopIteration:
                            pass
                # Wo_conv over the DVE depthwise output
                for oc in range(NDA):
                    for tt in range(NTQ):
                        pc = ps_d.tile([128, 512], F32, tag='vd')
                        for c in range(NDA):
                            nc.tensor.matmul(
                                pc[:, :],
                                wocT_s[:, c, oc * 128:(oc + 1) * 128],
                                cdw_s[:, c, tt * 512:(tt + 1) * 512],
                                start=(c == 0), stop=(c == NDA - 1),
                                skip_group_check=True)
                            yield
                        nc.vector.tensor_scalar_add(
                            fT_s[:, NDA + oc, tt * 512:(tt + 1) * 512],
                            pc[:, :], cbe_s[:, oc:oc + 1])
                # Wf conv half, accumulated into resbf (token-major)
                for i in range(NQC):
                    for ot in range(2):
                        pfc = ps_d.tile([128, 512], F32, tag='vd')
                        for fc in range(NDA):
                            nc.tensor.matmul(
                                pfc[:, :],
                                fT_s[:, NDA + fc, i * 128:(i + 1) * 128],
                                wf_s[:, NDA + fc, ot * 512:(ot + 1) * 512],
                                start=(fc == 0), stop=(fc == NDA - 1),
                                skip_group_check=True)
                            yield
                        nc.vector.scalar_tensor_tensor(
                            resbf[:, i, ot * 512:(ot + 1) * 512],
                            pfc[:, :], 1.0,
                            resbf[:, i, ot * 512:(ot + 1) * 512],
                            AOp.mult, AOp.add)

            drip_gen = drip_steps()

            def drip(n=1):
                for _ in range(n):
                    try:
                        next(drip_gen)
                    except StopIteration:
                        return

            # K tg1 projection, emitted mid-head-0 once kh1 has arrived;
            # part 0 covers token tile 2, part 1 tile 3.
            def emit_ktg1(part):
                ti = 2 + part
                for oc in range(NDA):
                    pk = ps_d.tile([128, 512], F32, tag='vd',
                                   name=f'pk1_{ti}{oc}')
                    for dc in range(NDA):
                        nc.tensor.matmul(
                            pk[:, :],
                            wk_s[:, dc, oc * 128:(oc + 1) * 128],
                            k_in[:, dc, ti * 512:(ti + 1) * 512],
                            start=(dc == 0), stop=(dc == NDA - 1),
                            skip_group_check=True)
                    nc.vector.tensor_scalar_add(
                        k_s[:, oc, ti * 512:(ti + 1) * 512],
                        pk[:, :], bk_s[:, oc:oc + 1])

            prev = None  # (h, o_ps, pend) carried into the next head

            def _finish_head(ph, po_ps, ppend):
                for kcp, p in ppend:
                    drip(1)
                    _emit_pv(nc, po_ps, vaug, p, ph, kcp)
                # normalize: rows 0:64 = denominator, 64:128 = O
                php = (ph % 2) * 64
                phc = ph // 2
                rec = rp.tile([64, TQ], F32, tag='rec')
                nc.vector.reciprocal_approx_fast(rec[:, :], po_ps[0:64, :])
                nc.vector.tensor_tensor(
                    o_s[php:php + 64, phc, :], po_ps[64:128, :], rec[:, :],
                    AOp.mult)
                nc.vector.tensor_scalar_add(
                    o_s[php:php + 64, phc, :], o_s[php:php + 64, phc, :],
                    bv_fm[php:php + 64, phc:phc + 1])

            for h in range(H):
                hp = (h % 2) * 64
                hc = h // 2
                o_ps = ps_o.tile([128, TQ], F32, tag='o')
                pend = []  # pending PV p_sb tiles
                for kc in range(NKC):
                    s_ps = ps_s.tile([128, TQ], F32, tag='s')
                    for tt in range(NTQ):
                        nc.tensor.matmul(
                            s_ps[:, tt * 512:(tt + 1) * 512],
                            k_s[hp:hp + 64, hc, kc * 128:(kc + 1) * 128],
                            q_s[hp:hp + 64, hc,
                                tt * 512:(tt + 1) * 512],
                            start=True, stop=True)
                    p_sb = wp.tile([128, TQ], BF16, tag='p')
                    nc.scalar.activation(p_sb[:, :], s_ps[:, :], Exp,
                                         scale=0.125)
                    drip(2 if h == 0 else 1)
                    if h == 0 and kc in (5, 9):
                        emit_ktg1((kc - 5) // 4)
                    if kc == 1 and prev is not None:
                        _finish_head(*prev)
                        prev = None
                    pend.append((kc, p_sb))
                    if len(pend) > (3 if h == 0 else 2):
                        kcp, p = pend.pop(0)
                        _emit_pv(nc, o_ps, vaug, p, h, kcp)
                prev = (h, o_ps, pend)
            _finish_head(*prev)

            drip(8 * NDA + NDA * NTQ * NDA + NQC * NDA * 2)

        # ---------------- Wo_attn (stationary reused across tt) -------------
        with tc.tile_pool(name=R + 'ps_wo', bufs=2, space='PSUM') as ps_wo:
            for oc in range(NDA):
                pa = [ps_wo.tile([128, 512], F32, tag=f'wop{tt}',
                                 name=f'pa{oc}{tt}') for tt in range(NTQ)]
                for dc in range(NDA):
                    for tt in range(NTQ):
                        nc.tensor.matmul(
                            pa[tt][:, :],
                            woa_s[:, dc, oc * 128:(oc + 1) * 128],
                            o_s[:, dc, tt * 512:(tt + 1) * 512],
                            start=(dc == 0), stop=(dc == NDA - 1))
                for tt in range(NTQ):
                    nc.scalar.activation(
                        fT_s[:, oc, tt * 512:(tt + 1) * 512], pa[tt][:, :],
                        Identity, bias=boa_s[:, oc:oc + 1])

        # -------- Wf (attn half) + residual-in-PSUM + LayerNorm --------
        with tc.tile_pool(name=R + 'lnw', bufs=3) as lp, \
             tc.tile_pool(name=R + 'ps_f', bufs=3, space='PSUM') as ps_f:
            for i in range(NQC):
                pf = ps_f.tile([128, D], F32, tag='f')
                # residual (+bf+Wf_conv) enters PSUM via identity matmul
                for ot in range(2):
                    nc.tensor.matmul(
                        pf[:, ot * 512:(ot + 1) * 512],
                        id_s[:, :],
                        resbf[:, i, ot * 512:(ot + 1) * 512],
                        start=True, stop=False)
                for fc in range(NDA):
                    for ot in range(2):
                        nc.tensor.matmul(
                            pf[:, ot * 512:(ot + 1) * 512],
                            fT_s[:, fc, i * 128:(i + 1) * 128],
                            wf_s[:, fc, ot * 512:(ot + 1) * 512],
                            start=False, stop=(fc == NDA - 1))
                stats = lp.tile([128, 2, 6], F32, tag='st')
                nc.vector.bn_stats(stats[:, 0, :], pf[:, 0:512])
                nc.vector.bn_stats(stats[:, 1, :], pf[:, 512:1024])
                mv = lp.tile([128, 2], F32, tag='mv')
                nc.vector.bn_aggr(mv[:, :], stats[:, :, :])
                sd = lp.tile([128, 1], F32, tag='sd')
                nc.scalar.activation(sd[:, :], mv[:, 1:2], Sqrt,
                                     bias=eps_s[:, 0:1])
                rstd = lp.tile([128, 1], F32, tag='rs')
                nc.vector.reciprocal(rstd[:, :], sd[:, :])
                nm = lp.tile([128, 1], F32, tag='nm')
                nc.vector.scalar_tensor_tensor(nm[:, :], mv[:, 0:1], -1.0,
                                               rstd[:, :], AOp.mult, AOp.mult)
                t1 = lp.tile([128, D], BF16, tag='t1')
                nc.scalar.activation(t1[:, :], pf[:, :], Identity,
                                     bias=nm[:, 0:1], scale=rstd[:, 0:1])
                o1 = lp.tile([128, D], BF16, tag='o1')
                nc.vector.tensor_tensor(o1[:, :], t1[:, :], ga_bc[:, :],
                                        AOp.mult)
                o_sb = lp.tile([128, D], F32, tag='ob')
                nc.vector.tensor_tensor(o_sb[:, :], o1[:, :], be_bc[:, :],
                                        AOp.add)
                nc.sync.dma_start(out=_rows(t['out'], D, i * 128, 128),
                                   in_=o_sb[:, :])


def _emit_pv(nc, o_ps, vaug, p_sb, h, kc):
    for tt in range(NTQ):
        nc.tensor.matmul(
            o_ps[:, tt * 512:(tt + 1) * 512],
            vaug[:, kc, h, :],
            p_sb[:, tt * 512:(tt + 1) * 512],
            start=(kc == 0), stop=(kc == NKC - 1),
            skip_group_check=True)


def make_in_maps(inputs):
    q = np.ascontiguousarray(np.asarray(inputs['queries'], np.float32))
    k = np.ascontiguousarray(np.asarray(inputs['keys'], np.float32))
    v = np.ascontiguousarray(np.asarray(inputs['values'], np.float32))
    W = {n: np.ascontiguousarray(np.asarray(inputs[n], np.float32).T)
         for n in ('Wq', 'Wk', 'Wv', 'Wo_attn', 'Wo_conv', 'Wf')}
    woc = np.asarray(inputs['Wo_conv'], np.float32)
    cbe = woc @ np.asarray(inputs['conv_b'], np.float32) + \
        np.asarray(inputs['bo_conv'], np.float32)
    com = {
        'wqT': W['Wq'].astype(BF), 'wkT': W['Wk'].astype(BF),
        'wvT': W['Wv'].astype(BF), 'woaT': W['Wo_attn'].astype(BF),
        'wocT': W['Wo_conv'].astype(BF), 'wfT': W['Wf'].astype(BF),
        'cw': np.asarray(inputs['conv_w'], np.float32).reshape(DC, 3),
        'bq': np.asarray(inputs['bq'], np.float32),
        'bk': np.asarray(inputs['bk'], np.float32),
        'bv': np.asarray(inputs['bv'], np.float32),
        'boa': np.asarray(inputs['bo_attn'], np.float32),
        'cbe': np.ascontiguousarray(cbe, dtype=np.float32),
        'gamma': np.asarray(inputs['gamma'], np.float32),
        'beta': np.asarray(inputs['beta'], np.float32),
        'ident': np.eye(128, dtype=BF),
    }
    com = {n: np.ascontiguousarray(a) for n, a in com.items()}
    in_maps = []
    for core in range(N_CORES):
        b, half = core // 2, core % 2
        r0, r1 = half * TQ, (half + 1) * TQ
        qc = np.zeros((TQ + 2, DC), np.float32)
        qc[1:TQ + 1] = q[b, r0:r1, DA:]
        if r0 > 0:
            qc[0] = q[b, r0 - 1, DA:]
        if r1 < L:
            qc[TQ + 1] = q[b, r1, DA:]
        m = dict(com)
        m['qaT'] = np.ascontiguousarray(q[b, r0:r1, :DA].T).astype(BF)
        m['qcT'] = np.ascontiguousarray(qc.T).astype(BF)
        m['qres'] = np.ascontiguousarray(
            q[b, r0:r1, :] + np.asarray(inputs['bf'], np.float32)).astype(BF)
        m['kT'] = np.ascontiguousarray(k[b, :, :DA].T).astype(BF)
        m['vT'] = np.ascontiguousarray(v[b, :, :DA].T).astype(BF)
        in_maps.append(m)
    return in_maps


_NC_CACHE = {}


def get_nc(reps=1):
    if reps not in _NC_CACHE:
        _NC_CACHE[reps] = build_nc(reps)
    return _NC_CACHE[reps]


def kernel(**inputs):
    from concourse.bass_utils import run_bass_kernel_spmd
    nc = get_nc(1)
    in_maps = make_in_maps(inputs)
    res = run_bass_kernel_spmd(nc, in_maps, core_ids=list(range(N_CORES)))
    out = np.empty((B, L, D), np.float32)
    for core in range(N_CORES):
        b, half = core // 2, core % 2
        out[b, half * TQ:(half + 1) * TQ, :] = res.results[core]['out']
    return out
